# revision 1
# baseline (speedup 1.0000x reference)
"""Trainium2 Bass kernel: ColumnParallelLinear + multi-adapter LoRA routing.

Computes out = x @ W^T + bias + B[aid[s]] @ (A[aid[s]] @ x[s]) for each token.

Distribution across 8 NeuronCores (one TRN2 chip):
  - base GEMM is tensor-parallel over d_out (sharding_hint): weight + bias
    sharded, each core emits out_base^T [512, S]
  - the LoRA delta is token-parallel: core i computes the delta for ITS
    1024-token slab across ALL d_out (A and B are tiny and replicated), so
    the rank-16 A-projection is computed once per token chip-wide instead
    of 8x replicated; no collectives needed — the host adds the two partial
    results while unsharding (out[s,o] = base[core o/512] + delta[core s/1024])
  - each core's token axis is ROTATED on the host so its own slab occupies
    the first two 512-token tiles; the xa matmuls then reuse the base
    x-strips already in SBUF (no extra x traffic, no prefetch stall), and
    the host un-rotates the base output during unsharding

Per-core kernel (all matmuls bf16, K=128 tiles, N=512 moving):
  - host pre-transposes x so the contraction dim lands on SBUF partitions
  - per-token adapter routing = precomputed {0,1} mask multiplied into the
    xa PSUM tile on the VectorE before the B_cat matmuls
  - bias is added during base PSUM->SBUF eviction (per-partition scalar add)
  - the 64 B_cat delta matmuls are drip-fed 2-per-base-m-tile so their
    PSUM-evict chain (ScalarE copy) never gates the PE
  - DMA emission is interleaved (w chunk k / x chunk k) with small leading
    chunks so the first matmul issues after ~256KB of DMA
"""

import os
import sys

import numpy as np

try:
    import ml_dtypes
except ImportError:  # pragma: no cover
    sys.path.insert(0, "/opt/trn_rl_repo")
    import ml_dtypes

_P = 128  # SBUF partitions / matmul tile edge
_NT = 512  # token tile (matmul moving free dim, one PSUM bank of fp32)
_LR = 128  # L * R = 8 * 16 adapter-rank rows
_N_CORES = 8

_NC_CACHE = {}
LAST_RESULTS = None  # BassKernelResults of the most recent run (for test.py)


def _import_concourse():
    try:
        import concourse  # noqa: F401
    except ImportError:  # pragma: no cover
        for p in ("/opt/trn_rl_repo", "/root/.axon_site/_ro/trn_rl_repo"):
            if os.path.isdir(p) and p not in sys.path:
                sys.path.insert(0, p)


def build_nc(d_in: int, d_loc: int, s_tokens: int, s_own: int, d_out: int):
    """Build + finalize the per-core Bass kernel.

    d_loc: output features of this core's base shard
    s_own: tokens in this core's LoRA-delta slab (the FIRST s_own tokens of
           the core's rotated token order)
    d_out: full output width (the delta covers all of it)
    """
    _import_concourse()
    import concourse.tile as tile
    from concourse import bacc, mybir

    P, NT, LR = _P, _NT, _LR
    n_kt = d_in // P
    n_mt = d_loc // P
    n_nt = s_tokens // NT
    n_ot = s_own // NT  # own-slab token tiles
    n_dt = d_out // P  # delta feature tiles
    assert all(v % P == 0 for v in (d_in, d_loc, d_out)) and s_tokens % NT == 0
    assert s_own % NT == 0 and n_ot <= n_nt

    nc = bacc.Bacc("TRN2", target_bir_lowering=False, debug=False)

    bf16 = mybir.dt.bfloat16
    f32 = mybir.dt.float32

    xT = nc.dram_tensor("xT", [d_in, s_tokens], bf16, kind="ExternalInput").ap()
    w_t = nc.dram_tensor("w_t", [d_in, d_loc], bf16, kind="ExternalInput").ap()
    a_t = nc.dram_tensor("a_t", [d_in, LR], bf16, kind="ExternalInput").ap()
    b_cat_t = nc.dram_tensor("b_cat_t", [LR, d_out], bf16, kind="ExternalInput").ap()
    mask_own = nc.dram_tensor("mask_own", [LR, s_own], bf16, kind="ExternalInput").ap()
    bias_pre = nc.dram_tensor("bias_pre", [P, n_mt], f32, kind="ExternalInput").ap()
    out_t = nc.dram_tensor("out_t", [d_loc, s_tokens], f32, kind="ExternalOutput").ap()
    delta_t = nc.dram_tensor("delta_t", [d_out, s_own], bf16, kind="ExternalOutput").ap()

    # [d_in, n] with d_in = kt*128 + p  ->  [p, kt, n]
    xT_v = xT.rearrange("(kt p) s -> p kt s", p=P)
    w_v = w_t.rearrange("(kt p) m -> p kt m", p=P)
    a_v = a_t.rearrange("(kt p) m -> p kt m", p=P)

    XCHUNK = 4  # k-tiles per x/w DMA chunk
    # finer chunks at the very start so the first matmul issues after ~256KB
    START_BOUNDS = [0, 1, 2, 3, 4]
    c = START_BOUNDS[-1]
    while c < n_kt:
        c = min(c + XCHUNK, n_kt)
        START_BOUNDS.append(c)
    START_BOUNDS = sorted(set(b for b in START_BOUNDS if b <= n_kt))

    with tile.TileContext(nc) as tc:
        with (
            tc.tile_pool(name="const", bufs=1) as const_pool,
            tc.tile_pool(name="xp", bufs=1) as x_pool,
            tc.tile_pool(name="outp", bufs=1) as out_pool,
            tc.tile_pool(name="psum", bufs=1, space="PSUM") as psum_pool,
        ):
            w_all = const_pool.tile([P, n_kt, d_loc], bf16)
            b_cat = const_pool.tile([P, n_dt, P], bf16)
            bias_sb = const_pool.tile([P, n_mt], f32)
            a_all = const_pool.tile([P, n_kt, LR], bf16)
            xa_sb = const_pool.tile([P, s_own], bf16)
            mask_sb = const_pool.tile([P, s_own], bf16)

            # Deferred LoRA-delta jobs, drip-fed between base m-tiles so the
            # PSUM-evict chain (ACT copy) never gates the PE.
            delta_jobs = []

            def emit_delta(k):
                for _ in range(k):
                    if not delta_jobs:
                        return
                    n, m = delta_jobs.pop(0)
                    dl_ps = psum_pool.tile(
                        [P, NT], f32, tag="dl", bufs=2, name=f"dl_ps{n}_{m}"
                    )
                    nc.tensor.matmul(
                        dl_ps[:],
                        b_cat[:, m, :],
                        xa_sb[:, n * NT : (n + 1) * NT],
                        start=True,
                        stop=True,
                    )
                    d_sb = out_pool.tile(
                        [P, NT], bf16, tag="d_sb", bufs=4, name=f"d_sb{n}_{m}"
                    )
                    nc.scalar.copy(d_sb[:], dl_ps[:])
                    nc.sync.dma_start(
                        delta_t[m * P : (m + 1) * P, n * NT : (n + 1) * NT], d_sb[:]
                    )

            def load_x_strip(j):
                x_strip = x_pool.tile(
                    [P, n_kt, NT], bf16, tag="x_strip", bufs=3, name=f"x_strip{j}"
                )
                tok = slice(j * NT, (j + 1) * NT)
                for c in range(0, n_kt, XCHUNK):
                    e = min(c + XCHUNK, n_kt)
                    nc.sync.dma_start(x_strip[:, c:e, :], xT_v[:, c:e, tok])
                return x_strip

            def evict_base(j, m, ps):
                tok0 = j * NT
                o_sb = out_pool.tile(
                    [P, NT], f32, tag="o_sb", bufs=6, name=f"o_sb{j}_{m}"
                )
                nc.vector.tensor_scalar_add(
                    out=o_sb[:], in0=ps[:], scalar1=bias_sb[:, m : m + 1]
                )
                # the very last evict+store trails the final matmul: spread it
                # over several DMA engines so the kernel tail shrinks
                pieces = {n_mt - 2: 2, n_mt - 1: 4}.get(m, 1) if j == n_nt - 1 else 1
                step = NT // pieces
                for q in range(pieces):
                    nc.sync.dma_start(
                        out_t[
                            m * P : (m + 1) * P,
                            tok0 + q * step : tok0 + (q + 1) * step,
                        ],
                        o_sb[:, q * step : (q + 1) * step],
                    )
                emit_delta(2)

            def base_ntile(j, x_strip, k_outer=False):
                if not k_outer:
                    for m in range(n_mt):
                        ps = psum_pool.tile(
                            [P, NT], f32, tag="base", bufs=4, name=f"ps{j}_{m}"
                        )
                        for kt in range(n_kt):
                            nc.tensor.matmul(
                                ps[:],
                                w_all[:, kt, m * P : (m + 1) * P],
                                x_strip[:, kt, :],
                                start=(kt == 0),
                                stop=(kt == n_kt - 1),
                            )
                        evict_base(j, m, ps)
                    return
                # k-outer: consume each k-chunk with one MM per m-tile the
                # moment it lands, so the DMA-paced prefix keeps the PE fed;
                # all n_mt PSUM banks accumulate concurrently
                pss = [
                    psum_pool.tile([P, NT], f32, tag="base", bufs=4, name=f"ps{j}_{m}")
                    for m in range(n_mt)
                ]
                for c, e in zip(START_BOUNDS, START_BOUNDS[1:]):
                    for m in range(n_mt):
                        for kt in range(c, e):
                            nc.tensor.matmul(
                                pss[m][:],
                                w_all[:, kt, m * P : (m + 1) * P],
                                x_strip[:, kt, :],
                                start=(kt == 0),
                                stop=(kt == n_kt - 1),
                            )
                for m in range(n_mt):
                    evict_base(j, m, pss[m])

            def xa_block(n, x_strip):
                # xa = A_all @ x^T for own-slab tile n, masked per-token;
                # queues that tile's 32 B_cat delta matmuls
                xa_ps = psum_pool.tile([P, NT], f32, tag="xa", bufs=2, name=f"xa_ps{n}")
                for kt in range(n_kt):
                    nc.tensor.matmul(
                        xa_ps[:],
                        a_all[:, kt, :],
                        x_strip[:, kt, :],
                        start=(kt == 0),
                        stop=(kt == n_kt - 1),
                    )
                nc.vector.tensor_mul(
                    out=xa_sb[:, n * NT : (n + 1) * NT],
                    in0=xa_ps[:],
                    in1=mask_sb[:, n * NT : (n + 1) * NT],
                )
                delta_jobs.extend((n, m) for m in range(n_dt))

            # ---- startup: interleave w chunks with x-strip j=0 chunks so the
            # first base matmuls have their operands after ~128KB of DMA;
            # the leading single-k-tile chunks are split in half across two
            # DMA engines to halve their arrival latency
            x_strip0 = x_pool.tile(
                [P, n_kt, NT], bf16, tag="x_strip", bufs=3, name="x_strip_first"
            )
            for c, e in zip(START_BOUNDS, START_BOUNDS[1:]):
                if e - c == 1:
                    q = d_loc // 4 if c == 0 else d_loc // 2
                    for h in range(0, d_loc, q):
                        nc.sync.dma_start(w_all[:, c, h : h + q], w_v[:, c, h : h + q])
                    q = NT // 4 if c == 0 else NT // 2
                    for h in range(0, NT, q):
                        nc.sync.dma_start(
                            x_strip0[:, c, h : h + q], xT_v[:, c, h : h + q]
                        )
                else:
                    nc.sync.dma_start(w_all[:, c:e, :], w_v[:, c:e, :])
                    nc.sync.dma_start(x_strip0[:, c:e, :], xT_v[:, c:e, 0:NT])
            nc.sync.dma_start(bias_sb[:], bias_pre)
            # warm the strip prefetch pipeline before any compute is emitted
            # (fresh pool slots -> these issue immediately on the Sync engine)
            strips = {0: x_strip0}
            K_OUTER = set()
            for j in (1, 2):
                if j < n_nt:
                    strips[j] = load_x_strip(j)
            # LoRA constants (a few MB; needed from ~40us in)
            for c in range(0, n_kt, XCHUNK):
                e = min(c + XCHUNK, n_kt)
                nc.sync.dma_start(a_all[:, c:e, :], a_v[:, c:e, :])
            nc.sync.dma_start(mask_sb[:], mask_own)
            for c in range(n_dt):
                nc.sync.dma_start(b_cat[:, c, :], b_cat_t[:, c * P : (c + 1) * P])

            for j in range(n_nt):
                x_strip = strips.pop(j) if j in strips else load_x_strip(j)
                base_ntile(j, x_strip, k_outer=j in K_OUTER)
                if j < n_ot:
                    xa_block(j, x_strip)
            while delta_jobs:
                emit_delta(len(delta_jobs))

    nc.finalize()
    return nc


def _get_nc(key):
    if key not in _NC_CACHE:
        _NC_CACHE[key] = build_nc(*key)
    return _NC_CACHE[key]


def make_in_maps(x, adapter_ids, weight, bias, A_buffer, B_buffer, n_cores=_N_CORES):
    """Host-side shard + layout prep. Returns (in_maps, shapes)."""
    bf16 = ml_dtypes.bfloat16
    x = np.asarray(x, dtype=np.float32)
    adapter_ids = np.asarray(adapter_ids, dtype=np.int32)
    weight = np.asarray(weight, dtype=np.float32)
    bias = np.asarray(bias, dtype=np.float32)
    A_buffer = np.asarray(A_buffer, dtype=np.float32)
    B_buffer = np.asarray(B_buffer, dtype=np.float32)

    S, D_IN = x.shape
    D_OUT = weight.shape[0]
    L, R, _ = A_buffer.shape
    d_loc = D_OUT // n_cores
    s_own = S // n_cores
    LR = L * R
    assert LR == _LR

    xT = np.ascontiguousarray(x.astype(bf16).T)  # [D_IN, S]
    a_t = np.ascontiguousarray(A_buffer.reshape(LR, D_IN).astype(bf16).T)
    b_cat_t = np.ascontiguousarray(
        B_buffer.transpose(0, 2, 1).reshape(LR, D_OUT).astype(bf16)
    )
    maskT = (np.arange(LR)[:, None] // R == adapter_ids[None, :]).astype(bf16)

    in_maps = []
    for i in range(n_cores):
        osl = slice(i * d_loc, (i + 1) * d_loc)
        w_t = np.ascontiguousarray(weight[osl].astype(bf16).T)  # [D_IN, d_loc]
        bias_pre = np.ascontiguousarray(bias[osl].reshape(d_loc // _P, _P).T)
        # rotate the token axis so core i's own slab comes first
        xT_rot = np.roll(xT, -i * s_own, axis=1) if i else xT
        in_maps.append(
            {
                "xT": np.ascontiguousarray(xT_rot),
                "w_t": w_t,
                "a_t": a_t,
                "b_cat_t": b_cat_t,
                "mask_own": np.ascontiguousarray(
                    maskT[:, i * s_own : (i + 1) * s_own]
                ),
                "bias_pre": bias_pre,
            }
        )
    return in_maps, (S, D_IN, D_OUT, d_loc, s_own)


def kernel(x, adapter_ids, weight, bias, A_buffer, B_buffer):
    global LAST_RESULTS
    _import_concourse()
    from concourse.bass_utils import run_bass_kernel_spmd

    in_maps, (S, D_IN, D_OUT, d_loc, s_own) = make_in_maps(
        x, adapter_ids, weight, bias, A_buffer, B_buffer
    )
    nc = _get_nc((D_IN, d_loc, S, s_own, D_OUT))
    LAST_RESULTS = run_bass_kernel_spmd(nc, in_maps, core_ids=list(range(_N_CORES)))
    res = LAST_RESULTS.results
    out = np.empty((S, D_OUT), dtype=np.float32)
    for i in range(_N_CORES):
        # un-rotate this core's token axis while scattering its base shard
        base = res[i]["out_t"]
        if i:
            base = np.roll(base, i * s_own, axis=1)
        out[:, i * d_loc : (i + 1) * d_loc] = base.T
    for i in range(_N_CORES):
        out[i * s_own : (i + 1) * s_own, :] += res[i]["delta_t"].T.astype(np.float32)
    return out



# revision 5
# speedup vs baseline: 1.1889x; 1.1889x over previous
"""Trainium2 Bass kernel: ColumnParallelLinear + multi-adapter LoRA routing.

Computes out = x @ W^T + bias + B[aid[s]] @ (A[aid[s]] @ x[s]) for each token.

Distribution across 8 NeuronCores (one TRN2 chip):
  - base GEMM is tensor-parallel over d_out (sharding_hint): weight + bias
    sharded, each core emits out_base^T [512, S]
  - the LoRA delta is token-parallel: core i computes the delta for ITS
    1024-token slab across ALL d_out (A and B are tiny and replicated), so
    the rank-16 A-projection is computed once per token chip-wide instead
    of 8x replicated; no collectives needed — the host adds the two partial
    results while unsharding (out[s,o] = base[core o/512] + delta[core s/1024])
  - each core's token axis is ROTATED on the host so its own slab occupies
    the first two 512-token tiles; the xa matmuls then reuse the base
    x-strips already in SBUF (no extra x traffic, no prefetch stall), and
    the host un-rotates the base output during unsharding

Per-core kernel (all matmuls bf16, K=128 tiles, N=512 moving):
  - host pre-transposes x so the contraction dim lands on SBUF partitions
  - per-token adapter routing = precomputed {0,1} mask multiplied into the
    xa PSUM tile on the VectorE before the B_cat matmuls
  - bias is added during base PSUM->SBUF eviction (per-partition scalar add)
  - base output is stored bf16 (halves the dominant store traffic; host
    upcasts) — well inside the error budget
  - DMA triggers are the scarce resource at startup (~0.7us each on a
    sequencer): startup w/x chunks are consolidated and dual-issued on the
    two HWDGE queues (Sync + Scalar); b_cat/A load as single descriptors
  - the 64 B_cat delta matmuls are drip-fed 2-per-base-m-tile; 4 PSUM
    banks for them so the drip never waits on the ScalarE evict chain;
    the last few are held back to form the kernel tail (small flushes)
"""

import os
import sys

import numpy as np

try:
    import ml_dtypes
except ImportError:  # pragma: no cover
    sys.path.insert(0, "/opt/trn_rl_repo")
    import ml_dtypes

_P = 128  # SBUF partitions / matmul tile edge
_NT = 512  # token tile (matmul moving free dim, one PSUM bank of fp32)
_LR = 128  # L * R = 8 * 16 adapter-rank rows
_N_CORES = 8

_NC_CACHE = {}
LAST_RESULTS = None  # BassKernelResults of the most recent run (for test.py)


def _import_concourse():
    try:
        import concourse  # noqa: F401
    except ImportError:  # pragma: no cover
        for p in ("/opt/trn_rl_repo", "/root/.axon_site/_ro/trn_rl_repo"):
            if os.path.isdir(p) and p not in sys.path:
                sys.path.insert(0, p)


def build_nc(d_in: int, d_loc: int, s_tokens: int, s_own: int, d_out: int):
    """Build + finalize the per-core Bass kernel.

    d_loc: output features of this core's base shard
    s_own: tokens in this core's LoRA-delta slab (the FIRST s_own tokens of
           the core's rotated token order)
    d_out: full output width (the delta covers all of it)
    """
    _import_concourse()
    import concourse.tile as tile
    from concourse import bacc, mybir

    P, NT, LR = _P, _NT, _LR
    n_kt = d_in // P
    n_mt = d_loc // P
    n_nt = s_tokens // NT
    n_ot = s_own // NT  # own-slab token tiles
    n_dt = d_out // P  # delta feature tiles
    assert all(v % P == 0 for v in (d_in, d_loc, d_out)) and s_tokens % NT == 0
    assert s_own % NT == 0 and n_ot <= n_nt

    nc = bacc.Bacc("TRN2", target_bir_lowering=False, debug=False)

    bf16 = mybir.dt.bfloat16
    f32 = mybir.dt.float32

    xT = nc.dram_tensor("xT", [d_in, s_tokens], bf16, kind="ExternalInput").ap()
    w_t = nc.dram_tensor("w_t", [d_in, d_loc], bf16, kind="ExternalInput").ap()
    a_t = nc.dram_tensor("a_t", [d_in, LR], bf16, kind="ExternalInput").ap()
    b_cat_t = nc.dram_tensor("b_cat_t", [LR, d_out], bf16, kind="ExternalInput").ap()
    mask_own = nc.dram_tensor("mask_own", [LR, s_own], bf16, kind="ExternalInput").ap()
    bias_pre = nc.dram_tensor("bias_pre", [P, n_mt], f32, kind="ExternalInput").ap()
    out_t = nc.dram_tensor("out_t", [d_loc, s_tokens], bf16, kind="ExternalOutput").ap()
    delta_t = nc.dram_tensor("delta_t", [d_out, s_own], bf16, kind="ExternalOutput").ap()

    # [d_in, n] with d_in = kt*128 + p  ->  [p, kt, n]
    xT_v = xT.rearrange("(kt p) s -> p kt s", p=P)
    w_v = w_t.rearrange("(kt p) m -> p kt m", p=P)
    a_v = a_t.rearrange("(kt p) m -> p kt m", p=P)
    b_v = b_cat_t.rearrange("p (c q) -> p c q", q=P)

    # startup w / x0 chunk boundaries: tiny first so the first matmul can
    # issue after ~256KB, growing so trigger count stays small
    START_BOUNDS = [0, 1, 2, 4, 8, 16, 24, n_kt]
    START_BOUNDS = sorted(set(b for b in START_BOUNDS if b <= n_kt))

    TAIL_JOBS = 4  # delta jobs reserved to run after the last base chain

    with tile.TileContext(nc) as tc:
        with (
            tc.tile_pool(name="const", bufs=1) as const_pool,
            tc.tile_pool(name="xp", bufs=1) as x_pool,
            tc.tile_pool(name="outp", bufs=1) as out_pool,
            tc.tile_pool(name="psum", bufs=1, space="PSUM") as psum_pool,
        ):
            w_all = const_pool.tile([P, n_kt, d_loc], bf16)
            b_cat = const_pool.tile([P, n_dt, P], bf16)
            bias_sb = const_pool.tile([P, n_mt], f32)
            a_all = const_pool.tile([P, n_kt, LR], bf16)
            xa_sb = const_pool.tile([P, s_own], bf16)
            mask_sb = const_pool.tile([P, s_own], bf16)

            # Deferred LoRA-delta jobs, drip-fed between base m-tiles so the
            # PSUM-evict chain (ACT copy) never gates the PE.
            delta_jobs = []

            def emit_delta(k, drain=False):
                for _ in range(k):
                    if len(delta_jobs) <= (0 if drain else TAIL_JOBS):
                        return
                    n, m = delta_jobs.pop(0)
                    dl_ps = psum_pool.tile(
                        [P, NT], f32, tag="dl", bufs=4, name=f"dl_ps{n}_{m}"
                    )
                    nc.tensor.matmul(
                        dl_ps[:],
                        b_cat[:, m, :],
                        xa_sb[:, n * NT : (n + 1) * NT],
                        start=True,
                        stop=True,
                    )
                    d_sb = out_pool.tile(
                        [P, NT], bf16, tag="d_sb", bufs=4, name=f"d_sb{n}_{m}"
                    )
                    nc.scalar.copy(d_sb[:], dl_ps[:])
                    nc.sync.dma_start(
                        delta_t[m * P : (m + 1) * P, n * NT : (n + 1) * NT], d_sb[:]
                    )

            def load_x_strip(j):
                # one whole-strip trigger (descriptors spray across engines)
                x_strip = x_pool.tile(
                    [P, n_kt, NT], bf16, tag="x_strip", bufs=3, name=f"x_strip{j}"
                )
                tok = slice(j * NT, (j + 1) * NT)
                nc.sync.dma_start(x_strip[:, :, :], xT_v[:, :, tok])
                return x_strip

            def evict_base(j, m, ps):
                o_sb = out_pool.tile(
                    [P, NT], bf16, tag="o_sb", bufs=6, name=f"o_sb{j}_{m}"
                )
                nc.vector.tensor_scalar_add(
                    out=o_sb[:], in0=ps[:], scalar1=bias_sb[:, m : m + 1]
                )
                nc.sync.dma_start(
                    out_t[m * P : (m + 1) * P, j * NT : (j + 1) * NT], o_sb[:]
                )
                emit_delta(2)

            def base_ntile(j, x_strip):
                for m in range(n_mt):
                    ps = psum_pool.tile(
                        [P, NT], f32, tag="base", bufs=4, name=f"ps{j}_{m}"
                    )
                    for kt in range(n_kt):
                        nc.tensor.matmul(
                            ps[:],
                            w_all[:, kt, m * P : (m + 1) * P],
                            x_strip[:, kt, :],
                            start=(kt == 0),
                            stop=(kt == n_kt - 1),
                        )
                    evict_base(j, m, ps)

            def xa_block(n, x_strip):
                # xa = A_all @ x^T for own-slab tile n, masked per-token;
                # queues that tile's 32 B_cat delta matmuls
                xa_ps = psum_pool.tile([P, NT], f32, tag="dl", bufs=4, name=f"xa_ps{n}")
                for kt in range(n_kt):
                    nc.tensor.matmul(
                        xa_ps[:],
                        a_all[:, kt, :],
                        x_strip[:, kt, :],
                        start=(kt == 0),
                        stop=(kt == n_kt - 1),
                    )
                nc.vector.tensor_mul(
                    out=xa_sb[:, n * NT : (n + 1) * NT],
                    in0=xa_ps[:],
                    in1=mask_sb[:, n * NT : (n + 1) * NT],
                )
                delta_jobs.extend((n, m) for m in range(n_dt))

            # ---- startup: interleave w chunk k / x0 chunk k, alternating
            # between the two HWDGE trigger queues (Sync + Scalar) so the
            # issue rate doubles and the first matmul starts after ~256KB
            x_strip0 = x_pool.tile(
                [P, n_kt, NT], bf16, tag="x_strip", bufs=3, name="x_strip_first"
            )
            for c, e in zip(START_BOUNDS, START_BOUNDS[1:]):
                nc.sync.dma_start(w_all[:, c:e, :], w_v[:, c:e, :])
                nc.sync.dma_start(x_strip0[:, c:e, :], xT_v[:, c:e, 0:NT])
            nc.sync.dma_start(bias_sb[:], bias_pre)
            # warm the strip prefetch pipeline before any compute is emitted
            strips = {0: x_strip0}
            for j in (1, 2):
                if j < n_nt:
                    strips[j] = load_x_strip(j)
            # LoRA constants (a few MB; needed from ~45us in) — one trigger each
            nc.sync.dma_start(a_all[:, :, :], a_v[:, :, :])
            nc.sync.dma_start(mask_sb[:], mask_own)
            nc.sync.dma_start(b_cat[:, :, :], b_v[:, :, :])

            for j in range(n_nt):
                x_strip = strips.pop(j) if j in strips else load_x_strip(j)
                base_ntile(j, x_strip)
                if j < n_ot:
                    xa_block(j, x_strip)
            while delta_jobs:
                emit_delta(len(delta_jobs), drain=True)

    nc.finalize()
    return nc


def _get_nc(key):
    if key not in _NC_CACHE:
        _NC_CACHE[key] = build_nc(*key)
    return _NC_CACHE[key]


def make_in_maps(x, adapter_ids, weight, bias, A_buffer, B_buffer, n_cores=_N_CORES):
    """Host-side shard + layout prep. Returns (in_maps, shapes)."""
    bf16 = ml_dtypes.bfloat16
    x = np.asarray(x, dtype=np.float32)
    adapter_ids = np.asarray(adapter_ids, dtype=np.int32)
    weight = np.asarray(weight, dtype=np.float32)
    bias = np.asarray(bias, dtype=np.float32)
    A_buffer = np.asarray(A_buffer, dtype=np.float32)
    B_buffer = np.asarray(B_buffer, dtype=np.float32)

    S, D_IN = x.shape
    D_OUT = weight.shape[0]
    L, R, _ = A_buffer.shape
    d_loc = D_OUT // n_cores
    s_own = S // n_cores
    LR = L * R
    assert LR == _LR

    xT = np.ascontiguousarray(x.astype(bf16).T)  # [D_IN, S]
    a_t = np.ascontiguousarray(A_buffer.reshape(LR, D_IN).astype(bf16).T)
    b_cat_t = np.ascontiguousarray(
        B_buffer.transpose(0, 2, 1).reshape(LR, D_OUT).astype(bf16)
    )
    maskT = (np.arange(LR)[:, None] // R == adapter_ids[None, :]).astype(bf16)

    in_maps = []
    for i in range(n_cores):
        osl = slice(i * d_loc, (i + 1) * d_loc)
        w_t = np.ascontiguousarray(weight[osl].astype(bf16).T)  # [D_IN, d_loc]
        bias_pre = np.ascontiguousarray(bias[osl].reshape(d_loc // _P, _P).T)
        # rotate the token axis so core i's own slab comes first
        xT_rot = np.roll(xT, -i * s_own, axis=1) if i else xT
        in_maps.append(
            {
                "xT": np.ascontiguousarray(xT_rot),
                "w_t": w_t,
                "a_t": a_t,
                "b_cat_t": b_cat_t,
                "mask_own": np.ascontiguousarray(
                    maskT[:, i * s_own : (i + 1) * s_own]
                ),
                "bias_pre": bias_pre,
            }
        )
    return in_maps, (S, D_IN, D_OUT, d_loc, s_own)


def kernel(x, adapter_ids, weight, bias, A_buffer, B_buffer):
    global LAST_RESULTS
    _import_concourse()
    from concourse.bass_utils import run_bass_kernel_spmd

    in_maps, (S, D_IN, D_OUT, d_loc, s_own) = make_in_maps(
        x, adapter_ids, weight, bias, A_buffer, B_buffer
    )
    nc = _get_nc((D_IN, d_loc, S, s_own, D_OUT))
    LAST_RESULTS = run_bass_kernel_spmd(nc, in_maps, core_ids=list(range(_N_CORES)))
    res = LAST_RESULTS.results
    out = np.empty((S, D_OUT), dtype=np.float32)
    for i in range(_N_CORES):
        # un-rotate this core's token axis while scattering its base shard
        base = res[i]["out_t"]
        if i:
            base = np.roll(base, i * s_own, axis=1)
        out[:, i * d_loc : (i + 1) * d_loc] = base.T.astype(np.float32)
    for i in range(_N_CORES):
        out[i * s_own : (i + 1) * s_own, :] += res[i]["delta_t"].T.astype(np.float32)
    return out


# revision 6
# speedup vs baseline: 1.3440x; 1.1305x over previous
"""Trainium2 Bass kernel: ColumnParallelLinear + multi-adapter LoRA routing.

Computes out = x @ W^T + bias + B[aid[s]] @ (A[aid[s]] @ x[s]) for each token.

Distribution across 8 NeuronCores (one TRN2 chip):
  - base GEMM is tensor-parallel over d_out: weight + bias sharded, each
    core emits out_base^T [512, S] (stored bf16, host upcasts)
  - the LoRA delta is token-parallel: core i computes the delta for ITS
    1024-token slab across ALL d_out; the host adds the two partials while
    unsharding; each core's token axis is ROTATED so its own slab occupies
    the first two 512-token tiles and reuses the base x-strips in SBUF

Per-core kernel:
  - mixed-precision contraction: the FIRST 2*_N8 k-tiles of every
    512-token contraction chain (base GEMM and the LoRA A-projection) run
    as _N8 fp8e4 DoubleRow pair-matmuls (two 128-k-tiles fused per PE
    instruction at ~1.8x bf16 throughput); the rest are bf16.  W and A are
    pre-scaled by 64 on the host so fp8e4 stays in its normal range; the
    1/64 is folded into the PSUM eviction (base) and into the routing mask
    values (LoRA).  Offline-simulated rel err ~1.6e-2 vs the 2e-2 budget
    (the offline sim matched hardware to 7 digits on the all-bf16 build).
  - per-token adapter routing = precomputed {0, 1/64} mask multiplied into
    the xa PSUM tile on the VectorE before the B_cat matmuls
  - base eviction: out = psum*(1/64) + bias as one VectorE tensor_scalar
  - DMA triggers are the scarce resource (~0.7us each on the Sync
    sequencer): whole strips / constants load as single sprayed triggers
  - the 64 B_cat delta matmuls drip 2-per-base-m-tile through 4 PSUM
    banks; the last few are held back to form a short kernel tail
"""

import os
import sys

import numpy as np

try:
    import ml_dtypes
except ImportError:  # pragma: no cover
    sys.path.insert(0, "/opt/trn_rl_repo")
    import ml_dtypes

_P = 128  # SBUF partitions / matmul tile edge
_NT = 512  # token tile (matmul moving free dim, one PSUM bank of fp32)
_LR = 128  # L * R = 8 * 16 adapter-rank rows
_N8 = 4  # fp8 DoubleRow k-tile PAIRS at the head of each contraction chain
_WSCALE = 64.0  # host pre-scale on W and A so fp8e4 avoids subnormals
_N_CORES = 8

_NC_CACHE = {}
LAST_RESULTS = None  # BassKernelResults of the most recent run (for test.py)


def _import_concourse():
    try:
        import concourse  # noqa: F401
    except ImportError:  # pragma: no cover
        for p in ("/opt/trn_rl_repo", "/root/.axon_site/_ro/trn_rl_repo"):
            if os.path.isdir(p) and p not in sys.path:
                sys.path.insert(0, p)


def build_nc(d_in: int, d_loc: int, s_tokens: int, s_own: int, d_out: int):
    """Build + finalize the per-core Bass kernel."""
    _import_concourse()
    import concourse.tile as tile
    from concourse import bacc, mybir

    P, NT, LR, N8 = _P, _NT, _LR, _N8
    n_kt = d_in // P
    n_kb = n_kt - 2 * N8  # bf16 k-tiles (after the fp8 pairs)
    n_mt = d_loc // P
    n_nt = s_tokens // NT
    n_ot = s_own // NT  # own-slab token tiles
    n_dt = d_out // P  # delta feature tiles
    assert all(v % P == 0 for v in (d_in, d_loc, d_out)) and s_tokens % NT == 0
    assert s_own % NT == 0 and n_ot <= n_nt and 0 < 2 * N8 < n_kt

    nc = bacc.Bacc("TRN2", target_bir_lowering=False, debug=False)

    bf16 = mybir.dt.bfloat16
    f32 = mybir.dt.float32
    f8 = mybir.dt.float8e4
    DR = mybir.MatmulPerfMode.DoubleRow

    # bf16 operand parts hold k-tiles 2*N8..n_kt-1 of the contraction
    xT = nc.dram_tensor("xT", [n_kb * P, s_tokens], bf16, kind="ExternalInput").ap()
    w_t = nc.dram_tensor("w_t", [n_kb * P, d_loc], bf16, kind="ExternalInput").ap()
    a_t = nc.dram_tensor("a_t", [n_kb * P, LR], bf16, kind="ExternalInput").ap()
    # fp8 pair-interleaved parts hold k-tiles 0..2*N8-1
    x8T = nc.dram_tensor("x8T", [P, N8 * 2 * s_tokens], f8, kind="ExternalInput").ap()
    w8_t = nc.dram_tensor("w8_t", [P, N8 * 2 * d_loc], f8, kind="ExternalInput").ap()
    a8_t = nc.dram_tensor("a8_t", [P, N8 * 2 * LR], f8, kind="ExternalInput").ap()
    b_cat_t = nc.dram_tensor("b_cat_t", [LR, d_out], bf16, kind="ExternalInput").ap()
    mask_own = nc.dram_tensor("mask_own", [LR, s_own], bf16, kind="ExternalInput").ap()
    bias_pre = nc.dram_tensor("bias_pre", [P, n_mt], f32, kind="ExternalInput").ap()
    out_t = nc.dram_tensor("out_t", [d_loc, s_tokens], bf16, kind="ExternalOutput").ap()
    delta_t = nc.dram_tensor("delta_t", [d_out, s_own], bf16, kind="ExternalOutput").ap()

    # [n_kb*P, n] with row = kt*128 + p  ->  [p, kt, n]
    xT_v = xT.rearrange("(kt p) s -> p kt s", p=P)
    w_v = w_t.rearrange("(kt p) m -> p kt m", p=P)
    a_v = a_t.rearrange("(kt p) m -> p kt m", p=P)
    x8_v = x8T.rearrange("p (c i s) -> p c i s", c=N8, i=2)
    w8_v = w8_t.rearrange("p (c i m) -> p c i m", c=N8, i=2)
    a8_v = a8_t.rearrange("p (c i m) -> p c i m", c=N8, i=2)
    b_v = b_cat_t.rearrange("p (c q) -> p c q", q=P)

    # startup w / x0 bf16 chunk boundaries (in 0..n_kb space)
    START_BOUNDS = [0, 1, 2, 4, 8, 16, n_kb]
    START_BOUNDS = sorted(set(b for b in START_BOUNDS if b <= n_kb))

    TAIL_JOBS = 4  # delta jobs reserved to run after the last base chain
    UNSCALE = 1.0 / _WSCALE

    with tile.TileContext(nc) as tc:
        with (
            tc.tile_pool(name="const", bufs=1) as const_pool,
            tc.tile_pool(name="xp", bufs=1) as x_pool,
            tc.tile_pool(name="outp", bufs=1) as out_pool,
            tc.tile_pool(name="psum", bufs=1, space="PSUM") as psum_pool,
        ):
            w_all = const_pool.tile([P, n_kb, d_loc], bf16)
            w8_all = const_pool.tile([P, N8, 2, d_loc], f8)
            b_cat = const_pool.tile([P, n_dt, P], bf16)
            bias_sb = const_pool.tile([P, n_mt], f32)
            a_all = const_pool.tile([P, n_kb, LR], bf16)
            a8_all = const_pool.tile([P, N8, 2, LR], f8)
            xa_sb = const_pool.tile([P, s_own], bf16)
            mask_sb = const_pool.tile([P, s_own], bf16)

            delta_jobs = []

            def emit_delta(k, drain=False):
                for _ in range(k):
                    if len(delta_jobs) <= (0 if drain else TAIL_JOBS):
                        return
                    n, m = delta_jobs.pop(0)
                    dl_ps = psum_pool.tile(
                        [P, NT], f32, tag="dl", bufs=4, name=f"dl_ps{n}_{m}"
                    )
                    nc.tensor.matmul(
                        dl_ps[:],
                        b_cat[:, m, :],
                        xa_sb[:, n * NT : (n + 1) * NT],
                        start=True,
                        stop=True,
                    )
                    d_sb = out_pool.tile(
                        [P, NT], bf16, tag="d_sb", bufs=4, name=f"d_sb{n}_{m}"
                    )
                    nc.scalar.copy(d_sb[:], dl_ps[:])
                    nc.sync.dma_start(
                        delta_t[m * P : (m + 1) * P, n * NT : (n + 1) * NT], d_sb[:]
                    )

            def load_x_strip(j):
                # one bf16 + one fp8 trigger (descriptors spray across engines)
                x_strip = x_pool.tile(
                    [P, n_kb, NT], bf16, tag="x_strip", bufs=3, name=f"x_strip{j}"
                )
                x8_strip = x_pool.tile(
                    [P, N8, 2, NT], f8, tag="x8_strip", bufs=3, name=f"x8_strip{j}"
                )
                tok = slice(j * NT, (j + 1) * NT)
                nc.sync.dma_start(x_strip[:, :, :], xT_v[:, :, tok])
                nc.sync.dma_start(x8_strip[:, :, :, :], x8_v[:, :, :, tok])
                return x_strip, x8_strip

            def evict_base(j, m, ps):
                o_sb = out_pool.tile(
                    [P, NT], bf16, tag="o_sb", bufs=6, name=f"o_sb{j}_{m}"
                )
                nc.vector.tensor_scalar(
                    out=o_sb[:],
                    in0=ps[:],
                    scalar1=UNSCALE,
                    scalar2=bias_sb[:, m : m + 1],
                    op0=mybir.AluOpType.mult,
                    op1=mybir.AluOpType.add,
                )
                nc.sync.dma_start(
                    out_t[m * P : (m + 1) * P, j * NT : (j + 1) * NT], o_sb[:]
                )
                emit_delta(2)

            def chain(ps, stat8, stat16, x8_strip, x_strip):
                for c in range(N8):
                    nc.tensor.matmul(
                        ps[:],
                        stat8[:, c, :, :],
                        x8_strip[:, c, :, :],
                        start=(c == 0),
                        stop=False,
                        perf_mode=DR,
                    )
                for kt in range(n_kb):
                    nc.tensor.matmul(
                        ps[:],
                        stat16[:, kt, :],
                        x_strip[:, kt, :],
                        start=False,
                        stop=(kt == n_kb - 1),
                    )

            def base_ntile(j, x_strip, x8_strip):
                for m in range(n_mt):
                    ps = psum_pool.tile(
                        [P, NT], f32, tag="base", bufs=4, name=f"ps{j}_{m}"
                    )
                    chain(
                        ps,
                        w8_all[:, :, :, m * P : (m + 1) * P],
                        w_all[:, :, m * P : (m + 1) * P],
                        x8_strip,
                        x_strip,
                    )
                    evict_base(j, m, ps)

            def xa_block(n, x_strip, x8_strip):
                xa_ps = psum_pool.tile([P, NT], f32, tag="dl", bufs=4, name=f"xa_ps{n}")
                chain(xa_ps, a8_all, a_all, x8_strip, x_strip)
                # mask holds {0, 1/64}: routing and fp8 un-scaling in one mul
                nc.vector.tensor_mul(
                    out=xa_sb[:, n * NT : (n + 1) * NT],
                    in0=xa_ps[:],
                    in1=mask_sb[:, n * NT : (n + 1) * NT],
                )
                delta_jobs.extend((n, m) for m in range(n_dt))

            # ---- startup: fp8 parts first (small; they open every chain),
            # then interleaved bf16 w/x chunks with growing sizes
            x_strip0 = x_pool.tile(
                [P, n_kb, NT], bf16, tag="x_strip", bufs=3, name="x_strip_first"
            )
            x8_strip0 = x_pool.tile(
                [P, N8, 2, NT], f8, tag="x8_strip", bufs=3, name="x8_strip_first"
            )
            nc.sync.dma_start(w8_all[:, :, :, :], w8_v[:, :, :, :])
            nc.sync.dma_start(x8_strip0[:, :, :, :], x8_v[:, :, :, 0:NT])
            for c, e in zip(START_BOUNDS, START_BOUNDS[1:]):
                nc.sync.dma_start(w_all[:, c:e, :], w_v[:, c:e, :])
                nc.sync.dma_start(x_strip0[:, c:e, :], xT_v[:, c:e, 0:NT])
            nc.sync.dma_start(bias_sb[:], bias_pre)
            # warm the strip prefetch pipeline before any compute is emitted
            strips = {0: (x_strip0, x8_strip0)}
            for j in (1, 2):
                if j < n_nt:
                    strips[j] = load_x_strip(j)
            # LoRA constants (needed from ~45us in) — one trigger each
            nc.sync.dma_start(a_all[:, :, :], a_v[:, :, :])
            nc.sync.dma_start(a8_all[:, :, :, :], a8_v[:, :, :, :])
            nc.sync.dma_start(mask_sb[:], mask_own)
            nc.sync.dma_start(b_cat[:, :, :], b_v[:, :, :])

            for j in range(n_nt):
                x_strip, x8_strip = (
                    strips.pop(j) if j in strips else load_x_strip(j)
                )
                base_ntile(j, x_strip, x8_strip)
                if j < n_ot:
                    xa_block(j, x_strip, x8_strip)
            while delta_jobs:
                emit_delta(len(delta_jobs), drain=True)

    nc.finalize()
    return nc


def _get_nc(key):
    if key not in _NC_CACHE:
        _NC_CACHE[key] = build_nc(*key)
    return _NC_CACHE[key]


def _pair_pack(mat_t, n8, cols):
    """[2*n8*P, cols] (k-major rows) -> [P, n8, 2, cols] DoubleRow layout."""
    return np.ascontiguousarray(
        mat_t.reshape(n8, 2, _P, cols).transpose(2, 0, 1, 3)
    )


def make_in_maps(x, adapter_ids, weight, bias, A_buffer, B_buffer, n_cores=_N_CORES):
    """Host-side shard + layout prep. Returns (in_maps, shapes)."""
    bf16 = ml_dtypes.bfloat16
    f8 = ml_dtypes.float8_e4m3  # TRN FP8_EXP4: max normal 240
    x = np.asarray(x, dtype=np.float32)
    adapter_ids = np.asarray(adapter_ids, dtype=np.int32)
    weight = np.asarray(weight, dtype=np.float32)
    bias = np.asarray(bias, dtype=np.float32)
    A_buffer = np.asarray(A_buffer, dtype=np.float32)
    B_buffer = np.asarray(B_buffer, dtype=np.float32)

    S, D_IN = x.shape
    D_OUT = weight.shape[0]
    L, R, _ = A_buffer.shape
    d_loc = D_OUT // n_cores
    s_own = S // n_cores
    LR = L * R
    K8 = 2 * _N8 * _P  # contraction rows handled in fp8
    assert LR == _LR

    def q8(m):  # fp8 quantize with the TRN e4m3 clamp
        return np.clip(m, -240.0, 240.0).astype(f8)

    # fp8 part: k-rows 0..K8-1 ; bf16 part: k-rows K8..D_IN-1
    xT_b = np.ascontiguousarray(x[:, K8:].astype(bf16).T)  # [D_IN-K8, S]
    x8_p = _pair_pack(q8(x[:, :K8].T), _N8, S)  # [P, n8, 2, S]

    w64 = weight * _WSCALE
    a_cat = A_buffer.reshape(LR, D_IN).T * _WSCALE  # [D_IN, LR]
    a_t = np.ascontiguousarray(a_cat[K8:].astype(bf16))
    a8_p = _pair_pack(q8(a_cat[:K8]), _N8, LR)
    b_cat_t = np.ascontiguousarray(
        B_buffer.transpose(0, 2, 1).reshape(LR, D_OUT).astype(bf16)
    )
    maskT = (
        (np.arange(LR)[:, None] // R == adapter_ids[None, :]).astype(np.float32)
        / _WSCALE
    ).astype(bf16)

    in_maps = []
    for i in range(n_cores):
        osl = slice(i * d_loc, (i + 1) * d_loc)
        w_loc_t = w64[osl].T  # [D_IN, d_loc]
        w_t = np.ascontiguousarray(w_loc_t[K8:].astype(bf16))
        w8_p = _pair_pack(q8(w_loc_t[:K8]), _N8, d_loc)
        bias_pre = np.ascontiguousarray(bias[osl].reshape(d_loc // _P, _P).T)
        # rotate the token axis so core i's own slab comes first
        r = -i * s_own
        xT_rot = np.roll(xT_b, r, axis=1) if i else xT_b
        x8_rot = np.roll(x8_p, r, axis=3) if i else x8_p
        in_maps.append(
            {
                "xT": np.ascontiguousarray(xT_rot),
                "x8T": np.ascontiguousarray(x8_rot).reshape(_P, -1),
                "w_t": w_t,
                "w8_t": w8_p.reshape(_P, -1),
                "a_t": a_t,
                "a8_t": a8_p.reshape(_P, -1),
                "b_cat_t": b_cat_t,
                "mask_own": np.ascontiguousarray(
                    maskT[:, i * s_own : (i + 1) * s_own]
                ),
                "bias_pre": bias_pre,
            }
        )
    return in_maps, (S, D_IN, D_OUT, d_loc, s_own)


def kernel(x, adapter_ids, weight, bias, A_buffer, B_buffer):
    global LAST_RESULTS
    _import_concourse()
    from concourse.bass_utils import run_bass_kernel_spmd

    in_maps, (S, D_IN, D_OUT, d_loc, s_own) = make_in_maps(
        x, adapter_ids, weight, bias, A_buffer, B_buffer
    )
    nc = _get_nc((D_IN, d_loc, S, s_own, D_OUT))
    LAST_RESULTS = run_bass_kernel_spmd(nc, in_maps, core_ids=list(range(_N_CORES)))
    res = LAST_RESULTS.results
    out = np.empty((S, D_OUT), dtype=np.float32)
    for i in range(_N_CORES):
        # un-rotate this core's token axis while scattering its base shard
        base = res[i]["out_t"]
        if i:
            base = np.roll(base, i * s_own, axis=1)
        out[:, i * d_loc : (i + 1) * d_loc] = base.T.astype(np.float32)
    for i in range(_N_CORES):
        out[i * s_own : (i + 1) * s_own, :] += res[i]["delta_t"].T.astype(np.float32)
    return out


# revision 12
# speedup vs baseline: 1.3982x; 1.0403x over previous
"""Trainium2 Bass kernel: ColumnParallelLinear + multi-adapter LoRA routing.

Computes out = x @ W^T + bias + B[aid[s]] @ (A[aid[s]] @ x[s]) for each token.

Distribution across 8 NeuronCores (one TRN2 chip):
  - base GEMM is tensor-parallel over d_out: weight + bias sharded, each
    core emits out_base^T [512, S] (stored bf16, host upcasts)
  - the LoRA delta is token-parallel: core i computes the delta for ITS
    1024-token slab across ALL d_out; the host adds the two partials while
    unsharding; each core's token axis is ROTATED so its own slab occupies
    the first two 512-token tiles and reuses the base x-strips in SBUF

Per-core kernel:
  - mixed-precision contraction: the FIRST 2*_N8 k-tiles of every
    512-token contraction chain (base GEMM and the LoRA A-projection) run
    as _N8 fp8e4 DoubleRow pair-matmuls (two 128-k-tiles fused per PE
    instruction at ~1.8x bf16 throughput); the rest are bf16.  W and A are
    pre-scaled by 64 on the host so fp8e4 stays in its normal range; the
    1/64 is folded into the PSUM eviction (base) and into the routing mask
    values (LoRA).  Offline-simulated rel err ~1.6e-2 vs the 2e-2 budget
    (the offline sim matched hardware to 7 digits on the all-bf16 build).
  - per-token adapter routing = precomputed {0, 1/64} mask multiplied into
    the xa PSUM tile on the VectorE before the B_cat matmuls
  - base eviction: out = psum*(1/64) + bias as one VectorE tensor_scalar
  - DMA triggers are the scarce resource (~0.7us each on the Sync
    sequencer): whole strips / constants load as single sprayed triggers
  - the 64 B_cat delta matmuls drip 2-per-base-m-tile through 4 PSUM
    banks; the last few are held back to form a short kernel tail
"""

import os
import sys

import numpy as np

try:
    import ml_dtypes
except ImportError:  # pragma: no cover
    sys.path.insert(0, "/opt/trn_rl_repo")
    import ml_dtypes

_P = 128  # SBUF partitions / matmul tile edge
_NT = 512  # token tile (matmul moving free dim, one PSUM bank of fp32)
_LR = 128  # L * R = 8 * 16 adapter-rank rows
_N8 = 5  # fp8 DoubleRow k-tile PAIRS per contraction chain
_WSCALE = 64.0  # host pre-scale on W and A so fp8e4 avoids subnormals
_N_CORES = 8

_NC_CACHE = {}
LAST_RESULTS = None  # BassKernelResults of the most recent run (for test.py)


def _import_concourse():
    try:
        import concourse  # noqa: F401
    except ImportError:  # pragma: no cover
        for p in ("/opt/trn_rl_repo", "/root/.axon_site/_ro/trn_rl_repo"):
            if os.path.isdir(p) and p not in sys.path:
                sys.path.insert(0, p)


def build_nc(d_in: int, d_loc: int, s_tokens: int, s_own: int, d_out: int):
    """Build + finalize the per-core Bass kernel."""
    _import_concourse()
    import concourse.tile as tile
    from concourse import bacc, mybir

    P, NT, LR, N8 = _P, _NT, _LR, _N8
    n_kt = d_in // P
    n_kb = n_kt - 2 * N8  # bf16 k-tiles (after the fp8 pairs)
    n_mt = d_loc // P
    n_nt = s_tokens // NT
    n_ot = s_own // NT  # own-slab token tiles
    n_dt = d_out // P  # delta feature tiles
    assert all(v % P == 0 for v in (d_in, d_loc, d_out)) and s_tokens % NT == 0
    assert s_own % NT == 0 and n_ot <= n_nt and 0 < 2 * N8 < n_kt

    nc = bacc.Bacc("TRN2", target_bir_lowering=False, debug=False)

    bf16 = mybir.dt.bfloat16
    f32 = mybir.dt.float32
    f8 = mybir.dt.float8e4
    DR = mybir.MatmulPerfMode.DoubleRow

    # bf16 operand parts hold k-tiles 2*N8..n_kt-1 of the contraction
    xT = nc.dram_tensor("xT", [n_kb * P, s_tokens], bf16, kind="ExternalInput").ap()
    w_t = nc.dram_tensor("w_t", [n_kb * P, d_loc], bf16, kind="ExternalInput").ap()
    a_t = nc.dram_tensor("a_t", [n_kb * P, LR], bf16, kind="ExternalInput").ap()
    # fp8 pair-interleaved parts hold k-tiles 0..2*N8-1
    x8T = nc.dram_tensor("x8T", [P, N8 * 2 * s_tokens], f8, kind="ExternalInput").ap()
    w8_t = nc.dram_tensor("w8_t", [P, N8 * 2 * d_loc], f8, kind="ExternalInput").ap()
    a8_t = nc.dram_tensor("a8_t", [P, N8 * 2 * LR], f8, kind="ExternalInput").ap()
    b_cat_t = nc.dram_tensor("b_cat_t", [LR, d_out], bf16, kind="ExternalInput").ap()
    mask_own = nc.dram_tensor("mask_own", [LR, s_own], bf16, kind="ExternalInput").ap()
    bias_pre = nc.dram_tensor("bias_pre", [P, n_mt], f32, kind="ExternalInput").ap()
    out_t = nc.dram_tensor("out_t", [d_loc, s_tokens], bf16, kind="ExternalOutput").ap()
    delta_t = nc.dram_tensor("delta_t", [d_out, s_own], bf16, kind="ExternalOutput").ap()

    # [n_kb*P, n] with row = kt*128 + p  ->  [p, kt, n]
    xT_v = xT.rearrange("(kt p) s -> p kt s", p=P)
    w_v = w_t.rearrange("(kt p) m -> p kt m", p=P)
    a_v = a_t.rearrange("(kt p) m -> p kt m", p=P)
    x8_v = x8T.rearrange("p (c i s) -> p c i s", c=N8, i=2)
    w8_v = w8_t.rearrange("p (c i m) -> p c i m", c=N8, i=2)
    a8_v = a8_t.rearrange("p (c i m) -> p c i m", c=N8, i=2)
    b_v = b_cat_t.rearrange("p (c q) -> p c q", q=P)

    # startup w / x0 bf16 chunk boundaries (in 0..n_kb space)
    START_BOUNDS = [0, 1, 2, 4, 8, 16, n_kb]
    START_BOUNDS = sorted(set(b for b in START_BOUNDS if b <= n_kb))

    TAIL_JOBS = 2  # delta jobs reserved to run after the last base chain
    UNSCALE = 1.0 / _WSCALE

    with tile.TileContext(nc) as tc:
        with (
            tc.tile_pool(name="const", bufs=1) as const_pool,
            tc.tile_pool(name="xp", bufs=1) as x_pool,
            tc.tile_pool(name="outp", bufs=1) as out_pool,
            tc.tile_pool(name="psum", bufs=1, space="PSUM") as psum_pool,
        ):
            w_all = const_pool.tile([P, n_kb, d_loc], bf16)
            w8_all = const_pool.tile([P, N8, 2, d_loc], f8)
            b_cat = const_pool.tile([P, n_dt, P], bf16)
            bias_sb = const_pool.tile([P, n_mt], f32)
            a_all = const_pool.tile([P, n_kb, LR], bf16)
            a8_all = const_pool.tile([P, N8, 2, LR], f8)
            xa_sb = const_pool.tile([P, s_own], bf16)
            mask_sb = const_pool.tile([P, s_own], bf16)

            delta_jobs = []

            def emit_delta(k, drain=False):
                for _ in range(k):
                    if len(delta_jobs) <= (0 if drain else TAIL_JOBS):
                        return
                    n, m = delta_jobs.pop(0)
                    dl_ps = psum_pool.tile(
                        [P, NT], f32, tag="dl", bufs=4, name=f"dl_ps{n}_{m}"
                    )
                    nc.tensor.matmul(
                        dl_ps[:],
                        b_cat[:, m, :],
                        xa_sb[:, n * NT : (n + 1) * NT],
                        start=True,
                        stop=True,
                    )
                    d_sb = out_pool.tile(
                        [P, NT], bf16, tag="d_sb", bufs=4, name=f"d_sb{n}_{m}"
                    )
                    nc.vector.tensor_copy(d_sb[:], dl_ps[:])
                    nc.sync.dma_start(
                        delta_t[m * P : (m + 1) * P, n * NT : (n + 1) * NT], d_sb[:]
                    )

            def load_x_strip(j):
                # one bf16 + one fp8 trigger (descriptors spray across engines)
                x_strip = x_pool.tile(
                    [P, n_kb, NT], bf16, tag="x_strip", bufs=3, name=f"x_strip{j}"
                )
                x8_strip = x_pool.tile(
                    [P, N8, 2, NT], f8, tag="x8_strip", bufs=3, name=f"x8_strip{j}"
                )
                tok = slice(j * NT, (j + 1) * NT)
                nc.sync.dma_start(x_strip[:, :, :], xT_v[:, :, tok])
                nc.sync.dma_start(x8_strip[:, :, :, :], x8_v[:, :, :, tok])
                return x_strip, x8_strip

            def evict_base(j, m, ps):
                o_sb = out_pool.tile(
                    [P, NT], bf16, tag="o_sb", bufs=6, name=f"o_sb{j}_{m}"
                )
                nc.vector.tensor_scalar(
                    out=o_sb[:],
                    in0=ps[:],
                    scalar1=UNSCALE,
                    scalar2=bias_sb[:, m : m + 1],
                    op0=mybir.AluOpType.mult,
                    op1=mybir.AluOpType.add,
                )
                nc.sync.dma_start(
                    out_t[m * P : (m + 1) * P, j * NT : (j + 1) * NT], o_sb[:]
                )
                emit_delta(2)

            def chain(ps, stat8, stat16, x8_strip, x_strip):
                # bf16 k-tiles first: the chain can open on the first small
                # w/x chunk; the fp8 pair operands stream in meanwhile
                for kt in range(n_kb):
                    nc.tensor.matmul(
                        ps[:],
                        stat16[:, kt, :],
                        x_strip[:, kt, :],
                        start=(kt == 0),
                        stop=False,
                    )
                for c in range(N8):
                    nc.tensor.matmul(
                        ps[:],
                        stat8[:, c, :, :],
                        x8_strip[:, c, :, :],
                        start=False,
                        stop=(c == N8 - 1),
                        perf_mode=DR,
                    )

            def base_ntile(j, x_strip, x8_strip, after_m1=None):
                for m in range(n_mt):
                    ps = psum_pool.tile(
                        [P, NT], f32, tag="base", bufs=4, name=f"ps{j}_{m}"
                    )
                    chain(
                        ps,
                        w8_all[:, :, :, m * P : (m + 1) * P],
                        w_all[:, :, m * P : (m + 1) * P],
                        x8_strip,
                        x_strip,
                    )
                    evict_base(j, m, ps)
                    if m == 1 and after_m1 is not None:
                        after_m1()

            def xa_block(n, x_strip, x8_strip):
                xa_ps = psum_pool.tile([P, NT], f32, tag="dl", bufs=4, name=f"xa_ps{n}")
                chain(xa_ps, a8_all, a_all, x8_strip, x_strip)
                # mask holds {0, 1/64}: routing and fp8 un-scaling in one mul
                nc.vector.tensor_mul(
                    out=xa_sb[:, n * NT : (n + 1) * NT],
                    in0=xa_ps[:],
                    in1=mask_sb[:, n * NT : (n + 1) * NT],
                )
                delta_jobs.extend((n, m) for m in range(n_dt))

            # ---- startup: the chains open on bf16 k-tile 0, so its small
            # w/x chunks go first; the fp8 pair operands (needed ~5us later,
            # at each chain's tail) stream right behind them
            x_strip0 = x_pool.tile(
                [P, n_kb, NT], bf16, tag="x_strip", bufs=3, name="x_strip_first"
            )
            x8_strip0 = x_pool.tile(
                [P, N8, 2, NT], f8, tag="x8_strip", bufs=3, name="x8_strip_first"
            )
            for i, (c, e) in enumerate(zip(START_BOUNDS, START_BOUNDS[1:])):
                nc.sync.dma_start(w_all[:, c:e, :], w_v[:, c:e, :])
                nc.sync.dma_start(x_strip0[:, c:e, :], xT_v[:, c:e, 0:NT])
                if i == 1:
                    nc.sync.dma_start(w8_all[:, :, :, :], w8_v[:, :, :, :])
                    nc.sync.dma_start(x8_strip0[:, :, :, :], x8_v[:, :, :, 0:NT])
            nc.sync.dma_start(bias_sb[:], bias_pre)
            # prefetch strip j=1 now; j>=2 strips are emitted mid-tile (two
            # tiles ahead) so their transfers queue behind the startup bulk
            strips = {0: (x_strip0, x8_strip0)}
            if 1 < n_nt:
                strips[1] = load_x_strip(1)
            # LoRA constants (needed from ~40us in) — one trigger each
            nc.sync.dma_start(a_all[:, :, :], a_v[:, :, :])
            nc.sync.dma_start(a8_all[:, :, :, :], a8_v[:, :, :, :])
            nc.sync.dma_start(mask_sb[:], mask_own)
            nc.sync.dma_start(b_cat[:, :, :], b_v[:, :, :])

            for j in range(n_nt):
                x_strip, x8_strip = strips.pop(j)

                def prefetch(j=j):
                    if j + 2 < n_nt:
                        strips[j + 2] = load_x_strip(j + 2)

                base_ntile(j, x_strip, x8_strip, after_m1=prefetch)
                if j < n_ot:
                    xa_block(j, x_strip, x8_strip)
            while delta_jobs:
                emit_delta(len(delta_jobs), drain=True)

    nc.finalize()
    return nc


def _get_nc(key):
    if key not in _NC_CACHE:
        _NC_CACHE[key] = build_nc(*key)
    return _NC_CACHE[key]


def _pair_pack(mat_t, n8, cols):
    """[2*n8*P, cols] (k-major rows) -> [P, n8, 2, cols] DoubleRow layout."""
    return np.ascontiguousarray(
        mat_t.reshape(n8, 2, _P, cols).transpose(2, 0, 1, 3)
    )


def make_in_maps(x, adapter_ids, weight, bias, A_buffer, B_buffer, n_cores=_N_CORES):
    """Host-side shard + layout prep. Returns (in_maps, shapes)."""
    bf16 = ml_dtypes.bfloat16
    f8 = ml_dtypes.float8_e4m3  # TRN FP8_EXP4: max normal 240
    x = np.asarray(x, dtype=np.float32)
    adapter_ids = np.asarray(adapter_ids, dtype=np.int32)
    weight = np.asarray(weight, dtype=np.float32)
    bias = np.asarray(bias, dtype=np.float32)
    A_buffer = np.asarray(A_buffer, dtype=np.float32)
    B_buffer = np.asarray(B_buffer, dtype=np.float32)

    S, D_IN = x.shape
    D_OUT = weight.shape[0]
    L, R, _ = A_buffer.shape
    d_loc = D_OUT // n_cores
    s_own = S // n_cores
    LR = L * R
    K8 = 2 * _N8 * _P  # contraction rows handled in fp8
    assert LR == _LR

    def q8(m):  # fp8 quantize with the TRN e4m3 clamp
        return np.clip(m, -240.0, 240.0).astype(f8)

    # fp8 part: k-rows 0..K8-1 ; bf16 part: k-rows K8..D_IN-1
    xT_b = np.ascontiguousarray(x[:, K8:].astype(bf16).T)  # [D_IN-K8, S]
    x8_p = _pair_pack(q8(x[:, :K8].T), _N8, S)  # [P, n8, 2, S]

    w64 = weight * _WSCALE
    a_cat = A_buffer.reshape(LR, D_IN).T * _WSCALE  # [D_IN, LR]
    a_t = np.ascontiguousarray(a_cat[K8:].astype(bf16))
    a8_p = _pair_pack(q8(a_cat[:K8]), _N8, LR)
    b_cat_t = np.ascontiguousarray(
        B_buffer.transpose(0, 2, 1).reshape(LR, D_OUT).astype(bf16)
    )
    maskT = (
        (np.arange(LR)[:, None] // R == adapter_ids[None, :]).astype(np.float32)
        / _WSCALE
    ).astype(bf16)

    in_maps = []
    for i in range(n_cores):
        osl = slice(i * d_loc, (i + 1) * d_loc)
        w_loc_t = w64[osl].T  # [D_IN, d_loc]
        w_t = np.ascontiguousarray(w_loc_t[K8:].astype(bf16))
        w8_p = _pair_pack(q8(w_loc_t[:K8]), _N8, d_loc)
        bias_pre = np.ascontiguousarray(bias[osl].reshape(d_loc // _P, _P).T)
        # rotate the token axis so core i's own slab comes first
        r = -i * s_own
        xT_rot = np.roll(xT_b, r, axis=1) if i else xT_b
        x8_rot = np.roll(x8_p, r, axis=3) if i else x8_p
        in_maps.append(
            {
                "xT": np.ascontiguousarray(xT_rot),
                "x8T": np.ascontiguousarray(x8_rot).reshape(_P, -1),
                "w_t": w_t,
                "w8_t": w8_p.reshape(_P, -1),
                "a_t": a_t,
                "a8_t": a8_p.reshape(_P, -1),
                "b_cat_t": b_cat_t,
                "mask_own": np.ascontiguousarray(
                    maskT[:, i * s_own : (i + 1) * s_own]
                ),
                "bias_pre": bias_pre,
            }
        )
    return in_maps, (S, D_IN, D_OUT, d_loc, s_own)


def kernel(x, adapter_ids, weight, bias, A_buffer, B_buffer):
    global LAST_RESULTS
    _import_concourse()
    from concourse.bass_utils import run_bass_kernel_spmd

    in_maps, (S, D_IN, D_OUT, d_loc, s_own) = make_in_maps(
        x, adapter_ids, weight, bias, A_buffer, B_buffer
    )
    nc = _get_nc((D_IN, d_loc, S, s_own, D_OUT))
    LAST_RESULTS = run_bass_kernel_spmd(nc, in_maps, core_ids=list(range(_N_CORES)))
    res = LAST_RESULTS.results
    out = np.empty((S, D_OUT), dtype=np.float32)
    for i in range(_N_CORES):
        # un-rotate this core's token axis while scattering its base shard
        base = res[i]["out_t"]
        if i:
            base = np.roll(base, i * s_own, axis=1)
        out[:, i * d_loc : (i + 1) * d_loc] = base.T.astype(np.float32)
    for i in range(_N_CORES):
        out[i * s_own : (i + 1) * s_own, :] += res[i]["delta_t"].T.astype(np.float32)
    return out
